# revision 7
# baseline (speedup 1.0000x reference)
"""Trainium2 Bass kernel for nn_BranchMarkovLayer (gnn_message_passing).

Computation (per batch row b, node n of 64):
    data[b,n,:] = [ Zc[b,n,0:8], std(log1p(own[b,n])), std(log1p(par[b,n//8])),
                    std(log1p(root[b])) ]                       (11 features)
    h = relu(W1[n] @ data + b1[n]);  y = W2[n] @ h + b2[n]      (11 -> 6 -> 1)
    out = -12 + 24*sigmoid(0.2*y) = 12*tanh(0.1*(W2' h + b2'))  (W2' = 0.1*W2)

Standardization (mean/std over the FULL batch, ddof=1) is folded into W1/b1 on
the host, given per-column sums/sumsq computed on-device by a small stats NEFF.

Sharding: pure data-parallel over the batch axis across 8 NeuronCores.

Main NEFF per core (shard = 16384 rows):
  Phase A: load X cols [0:72) and [128:192), log1p on ACT, PE-transpose the
           74-feature blocks (root, par x8, own x64, ones) into a resident
           xT [74, 16384] SBUF tensor.
  Phase B: per 512-row tile: load Z [128,4,512], PE-transpose into zT [128f,512b]
           chunks, block-diag matmuls (float32r) for layer 1 (+x-part and bias
           via the ones row), relu on ACT, layer-2 matmuls accumulate into
           y psum [64,512], tanh(+b2 bias) on ACT, x12 + 32x32 block-transpose
           on DVE, strided DMA store to the natural [B, 64] layout.
"""

import numpy as np
from contextlib import ExitStack

N_CORES = 8
B_FULL = 131072
SHARD = B_FULL // N_CORES  # 16384
NN = 64      # nodes
NC = 8       # children per node
NH = 6       # hidden
NX = 74      # xT rows: root(1) + par(8) + own(64) + ones(1)

_cache = {}


def _dt():
    import concourse.mybir as mybir
    return mybir.dt


def _build_stats(rows):
    """NEFF 1: per-column sums and sum-of-squares of log1p over X columns
    [0:72) ("A": root@0, par@64..71) and [128:192) ("B": own)."""
    import concourse.mybir as mybir
    import concourse.tile as tile
    from concourse import bacc

    f32 = mybir.dt.float32
    Ln = mybir.ActivationFunctionType.Ln
    n_grp = rows // 1024

    nc = bacc.Bacc("TRN2", target_bir_lowering=False, debug=False,
                   num_devices=N_CORES)
    X = nc.dram_tensor("x", [rows, 192], f32, kind="ExternalInput").ap()
    OUT = nc.dram_tensor("stats", [128, 18], f32, kind="ExternalOutput").ap()

    # A-chunks of the flattened [8, 72] free block, then B-chunks of [8, 64]
    chunks_a = [(0, 0, 128), (1, 128, 128), (2, 256, 128), (3, 384, 128),
                (4, 512, 64)]
    chunks_b = [(5, 0, 128), (6, 128, 128), (7, 256, 128), (8, 384, 128)]

    with tile.TileContext(nc) as tc, ExitStack() as ctx:
        sb = ctx.enter_context(tc.tile_pool(name="sb", bufs=3))
        sb1 = ctx.enter_context(tc.tile_pool(name="sb1", bufs=1))
        ps = ctx.enter_context(tc.tile_pool(name="ps", bufs=2, space="PSUM"))

        ones = sb1.tile([128, 1], f32)
        nc.vector.memset(ones[:], 1.0)
        acc = sb1.tile([128, 18], f32)
        nc.vector.memset(acc[:], 0.0)

        Xv = X.rearrange("(g c p) f -> g p c f", p=128, c=8)
        for g in range(n_grp):
            tA = sb.tile([128, 8, 72], f32, tag="tA")
            nc.sync.dma_start(tA[:], Xv[g, :, :, 0:72])
            tB = sb.tile([128, 8, 64], f32, tag="tB")
            nc.sync.dma_start(tB[:], Xv[g, :, :, 128:192])
            loA = sb.tile([128, 8, 72], f32, tag="loA")
            nc.scalar.activation(loA[:], tA[:], Ln, bias=1.0)
            loB = sb.tile([128, 8, 64], f32, tag="loB")
            nc.scalar.activation(loB[:], tB[:], Ln, bias=1.0)
            sqA = sb.tile([128, 8, 72], f32, tag="sqA")
            nc.vector.tensor_mul(sqA[:], loA[:], loA[:])
            sqB = sb.tile([128, 8, 64], f32, tag="sqB")
            nc.vector.tensor_mul(sqB[:], loB[:], loB[:])

            loA_f = loA[:].rearrange("p c f -> p (c f)")
            loB_f = loB[:].rearrange("p c f -> p (c f)")
            sqA_f = sqA[:].rearrange("p c f -> p (c f)")
            sqB_f = sqB[:].rearrange("p c f -> p (c f)")
            ps_g = ps.tile([128, 18], f32, tag="ps_g")
            nc.vector.memset(ps_g[:], 0.0)
            for col, off, mc in chunks_a:
                nc.tensor.matmul(ps_g[0:mc, col:col + 1],
                                 loA_f[:, off:off + mc], ones[:])
                nc.tensor.matmul(ps_g[0:mc, 9 + col:10 + col],
                                 sqA_f[:, off:off + mc], ones[:])
            for col, off, mc in chunks_b:
                nc.tensor.matmul(ps_g[0:mc, col:col + 1],
                                 loB_f[:, off:off + mc], ones[:])
                nc.tensor.matmul(ps_g[0:mc, 9 + col:10 + col],
                                 sqB_f[:, off:off + mc], ones[:])
            nc.vector.tensor_add(acc[:], acc[:], ps_g[:])

        nc.sync.dma_start(OUT, acc[:])

    nc.compile()
    return nc


def _build_main(rows):
    """NEFF 2: the full forward pass given pre-folded weights."""
    import concourse.mybir as mybir
    import concourse.tile as tile
    from concourse import bacc
    from concourse.masks import make_identity

    f32 = mybir.dt.float32
    f32r = mybir.dt.float32r
    Ln = mybir.ActivationFunctionType.Ln
    Relu = mybir.ActivationFunctionType.Relu
    Tanh = mybir.ActivationFunctionType.Tanh
    n_grp = rows // 1024
    n_it = rows // 512

    nc = bacc.Bacc("TRN2", target_bir_lowering=False, debug=False,
                   num_devices=N_CORES)
    X = nc.dram_tensor("x", [rows, 192], f32, kind="ExternalInput").ap()
    Z = nc.dram_tensor("z", [rows, 512], f32, kind="ExternalInput").ap()
    WZ = nc.dram_tensor("wz", [4, 128, 96], f32r, kind="ExternalInput").ap()
    WX = nc.dram_tensor("wx", [4, NX, 96], f32r, kind="ExternalInput").ap()
    WH = nc.dram_tensor("wh", [4, 96, 64], f32r, kind="ExternalInput").ap()
    B2 = nc.dram_tensor("b2", [64, 1], f32, kind="ExternalInput").ap()
    Y = nc.dram_tensor("y", [rows, 64], f32, kind="ExternalOutput").ap()

    with tile.TileContext(nc) as tc, ExitStack() as ctx:
        cst = ctx.enter_context(tc.tile_pool(name="cst", bufs=1))
        ident = cst.tile([128, 128], f32)
        make_identity(nc, ident[:])
        wz_sb = cst.tile([128, 4, 96], f32r)
        nc.sync.dma_start(wz_sb[:], WZ.rearrange("g k m -> k g m"))
        wx_sb = cst.tile([NX, 4, 96], f32r)
        nc.sync.dma_start(wx_sb[:], WX.rearrange("g k m -> k g m"))
        wh_sb = cst.tile([96, 4, 64], f32r)
        nc.sync.dma_start(wh_sb[:], WH.rearrange("g k m -> k g m"))
        b2_sb = cst.tile([64, 1], f32)
        nc.sync.dma_start(b2_sb[:], B2)
        xT = cst.tile([NX, n_it, 512], f32r)  # resident log1p(x)^T (+ones row)

        # ---- Phase A: build xT ----
        with tc.tile_pool(name="pha", bufs=3) as pha, \
             tc.tile_pool(name="psA", bufs=2, space="PSUM") as psA:
            Xv = X.rearrange("(g c p) f -> g p c f", p=128, c=8)
            for g in range(n_grp):
                tA = pha.tile([128, 8, 72], f32, tag="tA")
                nc.sync.dma_start(tA[:], Xv[g, :, :, 0:72])
                tB = pha.tile([128, 8, 64], f32, tag="tB")
                nc.sync.dma_start(tB[:], Xv[g, :, :, 128:192])
                lo = pha.tile([128, 8, NX], f32, tag="lo")
                nc.scalar.activation(lo[:, :, 0:1], tA[:, :, 0:1], Ln, bias=1.0)
                nc.scalar.activation(lo[:, :, 1:9], tA[:, :, 64:72], Ln, bias=1.0)
                nc.scalar.activation(lo[:, :, 9:73], tB[:], Ln, bias=1.0)
                nc.vector.memset(lo[:, :, 73:74], 1.0)
                for half in range(2):
                    pt = psA.tile([NX, 4, 128], f32, tag="pt")
                    for cc in range(4):
                        c = half * 4 + cc
                        nc.tensor.transpose(pt[:, cc, :], lo[:, c, :], ident[:])
                    nc.vector.tensor_copy(
                        xT[:, 2 * g + half, :],
                        pt[:].rearrange("p c f -> p (c f)"))

        # ---- Phase B: main loop ----
        with tc.tile_pool(name="ztp", bufs=2) as ztp, \
             tc.tile_pool(name="zsp", bufs=6) as zsp, \
             tc.tile_pool(name="hsp", bufs=6) as hsp, \
             tc.tile_pool(name="ysp", bufs=3) as ysp, \
             tc.tile_pool(name="psZ", bufs=2, space="PSUM") as psZ, \
             tc.tile_pool(name="psH", bufs=3, space="PSUM") as psH, \
             tc.tile_pool(name="psY", bufs=2, space="PSUM") as psY:
            Zv = Z.rearrange("(i c p) f -> i p c f", p=128, c=4)
            Yv = Y.rearrange("(i c r) (a s) -> i a r c s", c=16, r=32, a=2, s=32)
            for it in range(n_it):
                zt = ztp.tile([128, 4, 512], f32, tag="zt")
                nc.sync.dma_start(zt[:], Zv[it])
                zs_tiles = []
                for fc in range(4):
                    pz = psZ.tile([128, 512], f32, tag="pz")
                    for c4 in range(4):
                        nc.tensor.transpose(
                            pz[:, c4 * 128:(c4 + 1) * 128],
                            zt[:, c4, fc * 128:(fc + 1) * 128], ident[:])
                    zst = zsp.tile([128, 512], f32r, tag="zs")
                    if fc % 2 == 0:
                        nc.vector.tensor_copy(zst[:], pz[:])
                    else:
                        nc.scalar.copy(zst[:], pz[:])
                    zs_tiles.append(zst)
                hs_tiles = []
                for g in range(4):
                    ph = psH.tile([96, 512], f32, tag="ph")
                    nc.tensor.matmul(ph[:], wz_sb[:, g, :],
                                     zs_tiles[g][:],
                                     start=True, stop=False)
                    nc.tensor.matmul(ph[:], wx_sb[:, g, :],
                                     xT[:, it, :],
                                     start=False, stop=True)
                    ht = hsp.tile([96, 512], f32r, tag="hs")
                    nc.scalar.activation(ht[:], ph[:], Relu)
                    hs_tiles.append(ht)
                py = psY.tile([64, 512], f32, tag="py")
                for g in range(4):
                    nc.tensor.matmul(py[:], wh_sb[:, g, :],
                                     hs_tiles[g][:],
                                     start=(g == 0), stop=(g == 3))
                ysb = ysp.tile([64, 512], f32, tag="ysb")
                nc.scalar.activation(ysb[:], py[:], Tanh, bias=b2_sb[:])
                y12 = ysp.tile([64, 512], f32, tag="y12")
                nc.vector.tensor_scalar_mul(y12[:], ysb[:], 12.0)
                ybt = ysp.tile([64, 512], f32, tag="ybt")
                nc.vector.transpose(ybt[:], y12[:])
                for a in range(2):
                    nc.sync.dma_start(
                        Yv[it, a],
                        ybt[32 * a:32 * a + 32, :].rearrange(
                            "r (c s) -> r c s", s=32))

    nc.compile()
    return nc


def _get_modules(rows=SHARD):
    key = ("mods", rows)
    if key not in _cache:
        _cache[key] = (_build_stats(rows), _build_main(rows))
    return _cache[key]


def _fold_weights(stats_list, W1, b1, W2, b2, rows_total):
    """Combine per-core stats, compute mu/sd, fold standardization into W1/b1,
    and build the device weight layouts."""
    a_flat = np.zeros(576, np.float64)
    b_flat = np.zeros(512, np.float64)
    a_sq = np.zeros(576, np.float64)
    b_sq = np.zeros(512, np.float64)
    chunks_a = [(0, 0, 128), (1, 128, 128), (2, 256, 128), (3, 384, 128),
                (4, 512, 64)]
    chunks_b = [(5, 0, 128), (6, 128, 128), (7, 256, 128), (8, 384, 128)]
    for st in stats_list:
        st = np.asarray(st, np.float64)
        for col, off, mc in chunks_a:
            a_flat[off:off + mc] += st[0:mc, col]
            a_sq[off:off + mc] += st[0:mc, col + 9]
        for col, off, mc in chunks_b:
            b_flat[off:off + mc] += st[0:mc, col]
            b_sq[off:off + mc] += st[0:mc, col + 9]
    s1A = a_flat.reshape(-1, 72).sum(0)
    s2A = a_sq.reshape(-1, 72).sum(0)
    s1B = b_flat.reshape(-1, 64).sum(0)
    s2B = b_sq.reshape(-1, 64).sum(0)
    n = float(rows_total)
    mu_root, mu_par, mu_own = s1A[0] / n, s1A[64:72] / n, s1B / n
    var = lambda s1, s2: (s2 - s1 * s1 / n) / (n - 1.0)
    sd_root = np.sqrt(var(s1A[0], s2A[0]))
    sd_par = np.sqrt(var(s1A[64:72], s2A[64:72]))
    sd_own = np.sqrt(var(s1B, s2B))

    W1 = np.asarray(W1, np.float64)
    b1 = np.asarray(b1, np.float64)
    W2 = np.asarray(W2, np.float64)
    b2 = np.asarray(b2, np.float64)
    Wz = W1[:, :, 0:8]
    Wown, Wpar, Wroot = W1[:, :, 8], W1[:, :, 9], W1[:, :, 10]
    par_idx = np.arange(NN) // 8
    Wown_f = Wown / sd_own[:, None]
    Wpar_f = Wpar / sd_par[par_idx][:, None]
    Wroot_f = Wroot / sd_root
    b1_f = (b1 - Wown * (mu_own / sd_own)[:, None]
            - Wpar * (mu_par / sd_par)[par_idx][:, None]
            - Wroot * (mu_root / sd_root))

    WZh = np.zeros((4, 128, 96), np.float32)
    WXh = np.zeros((4, NX, 96), np.float32)
    WHh = np.zeros((4, 96, 64), np.float32)
    for g in range(4):
        for nl in range(16):
            n_g = 16 * g + nl
            WZh[g, 8 * nl:8 * nl + 8, 6 * nl:6 * nl + 6] = Wz[n_g].T
            WXh[g, 0, 6 * nl:6 * nl + 6] = Wroot_f[n_g]
            WXh[g, 1 + n_g // 8, 6 * nl:6 * nl + 6] = Wpar_f[n_g]
            WXh[g, 9 + n_g, 6 * nl:6 * nl + 6] = Wown_f[n_g]
            WXh[g, 73, 6 * nl:6 * nl + 6] = b1_f[n_g]
            WHh[g, 6 * nl:6 * nl + 6, n_g] = 0.1 * W2[n_g, 0, :]
    B2h = (0.1 * b2).astype(np.float32).reshape(64, 1)
    return WZh, WXh, WHh, B2h


def kernel(**inputs):
    from concourse.bass_utils import run_bass_kernel_spmd

    X = np.ascontiguousarray(
        np.asarray(inputs["X_1tol"], np.float32).reshape(-1, 192))
    Z = np.ascontiguousarray(np.asarray(inputs["Z_l_next"], np.float32))
    rows_total = X.shape[0]
    shard = rows_total // N_CORES
    nc_stats, nc_main = _get_modules(shard)
    core_ids = list(range(N_CORES))

    in1 = [{"x": X[s * shard:(s + 1) * shard]} for s in range(N_CORES)]
    r1 = run_bass_kernel_spmd(nc_stats, in1, core_ids=core_ids)
    stats_list = [r1.results[s]["stats"] for s in range(N_CORES)]

    WZh, WXh, WHh, B2h = _fold_weights(
        stats_list, inputs["W1"], inputs["b1"], inputs["W2"], inputs["b2"],
        rows_total)

    in2 = [{"x": X[s * shard:(s + 1) * shard],
            "z": Z[s * shard:(s + 1) * shard],
            "wz": WZh, "wx": WXh, "wh": WHh, "b2": B2h}
           for s in range(N_CORES)]
    r2 = run_bass_kernel_spmd(nc_main, in2, core_ids=core_ids)
    out = np.concatenate([r2.results[s]["y"] for s in range(N_CORES)], axis=0)
    return out.astype(np.float32)


# revision 14
# speedup vs baseline: 1.6045x; 1.6045x over previous
"""Trainium2 Bass kernel for nn_BranchMarkovLayer (gnn_message_passing).

Computation (per batch row b, node n of 64):
    data[b,n,:] = [ Zc[b,n,0:8], std(log1p(own[b,n])), std(log1p(par[b,n//8])),
                    std(log1p(root[b])) ]                       (11 features)
    h = relu(W1[n] @ data + b1[n]);  y = W2[n] @ h + b2[n]      (11 -> 6 -> 1)
    out = -12 + 24*sigmoid(0.2*y) = 12*tanh(0.1*(W2' h + b2'))  (W2' = 0.1*W2)

Standardization (mean/std over the FULL batch, ddof=1) is folded into W1/b1 on
the host, given per-column sums/sumsq computed on-device by a small stats NEFF.

Sharding: pure data-parallel over the batch axis across 8 NeuronCores.

Main NEFF per core (shard = 16384 rows):
  Phase A: load X cols [0:72) and [128:192), log1p on ACT, PE-transpose the
           74-feature blocks (root, par x8, own x64, ones) into a resident
           xT [74, 16384] float32r SBUF tensor.
  Phase B: per 512-row tile: Z (pre-cast to bf16 and pre-transposed per shard
           on the host) is loaded as [128f, 2048b] tiles with plain large-burst
           DMAs (one per 16-node group per 4 iterations). Block-diagonal bf16
           matmuls for
           the layer-1 z-part accumulate with a float32r matmul for the x-part
           (+bias via the ones row) in PSUM [96, 512]; relu (split ACT/DVE)
           writes float32r; layer-2 float32r matmuls accumulate into y psum
           [64, 512]; tanh(+b2 bias) on ACT; x12 and a 32x32 block-transpose
           on DVE into a [64, 2048] staging tile; strided DMA store (128B
           bursts) to the natural [B, 64] layout every 4 iterations.
"""

import numpy as np
from concurrent.futures import ThreadPoolExecutor
from contextlib import ExitStack

N_CORES = 8
B_FULL = 131072
SHARD = B_FULL // N_CORES  # 16384
NN = 64      # nodes
NX = 74      # xT rows: root(1) + par(8) + own(64) + ones(1)

# A-chunks of the flattened [8, 72] free block, then B-chunks of [8, 64]
CHUNKS_A = [(0, 0, 128), (1, 128, 128), (2, 256, 128), (3, 384, 128),
            (4, 512, 64)]
CHUNKS_B = [(5, 0, 128), (6, 128, 128), (7, 256, 128), (8, 384, 128)]

_cache = {}


def _build_stats(rows):
    """NEFF 1: per-column sums and sum-of-squares of log1p over X columns
    [0:72) ("A": root@0, par@64..71) and [128:192) ("B": own).

    Per 1024-row group: log1p + square on ACT, accumulate into SBUF via DVE
    adds.  At the end, per-column sums via fp32 matmuls against a ones vector
    (contraction over the 128 batch partitions)."""
    import concourse.mybir as mybir
    import concourse.tile as tile
    from concourse import bacc

    f32 = mybir.dt.float32
    Ln = mybir.ActivationFunctionType.Ln
    Sq = mybir.ActivationFunctionType.Square
    n_grp = rows // 1024

    nc = bacc.Bacc("TRN2", target_bir_lowering=False, debug=False,
                   num_devices=N_CORES)
    X = nc.dram_tensor("x", [rows, 192], f32, kind="ExternalInput").ap()
    OUT = nc.dram_tensor("stats", [128, 18], f32, kind="ExternalOutput").ap()

    with tile.TileContext(nc) as tc, ExitStack() as ctx:
        sb = ctx.enter_context(tc.tile_pool(name="sb", bufs=3))
        sb1 = ctx.enter_context(tc.tile_pool(name="sb1", bufs=1))
        ps = ctx.enter_context(tc.tile_pool(name="ps", bufs=1, space="PSUM"))

        ones = sb1.tile([128, 1], f32)
        nc.vector.memset(ones[:], 1.0)
        acc_lo = sb1.tile([128, 8, 136], f32)   # [..., 0:72) = A, [72:136) = B
        nc.vector.memset(acc_lo[:], 0.0)
        acc_sq = sb1.tile([128, 8, 136], f32)
        nc.vector.memset(acc_sq[:], 0.0)

        Xv = X.rearrange("(g c p) f -> g p c f", p=128, c=8)
        for g in range(n_grp):
            tA = sb.tile([128, 8, 72], f32, tag="tA")
            nc.sync.dma_start(tA[:], Xv[g, :, :, 0:72])
            tB = sb.tile([128, 8, 64], f32, tag="tB")
            nc.sync.dma_start(tB[:], Xv[g, :, :, 128:192])
            lo = sb.tile([128, 8, 136], f32, tag="lo")
            nc.scalar.activation(lo[:, :, 0:72], tA[:], Ln, bias=1.0)
            nc.scalar.activation(lo[:, :, 72:136], tB[:], Ln, bias=1.0)
            sq = sb.tile([128, 8, 136], f32, tag="sq")
            nc.scalar.activation(sq[:], lo[:], Sq)
            nc.vector.tensor_add(acc_lo[:], acc_lo[:], lo[:])
            nc.vector.tensor_add(acc_sq[:], acc_sq[:], sq[:])

        # Final per-column sums: contract the 128 batch partitions on PE.
        # Flat layout: [8, 136] -> A cols at c*136+f (f<72), B at c*136+72+f.
        ps_t = ps.tile([128, 18], f32)
        acc_lo_f = acc_lo[:].rearrange("p c f -> p (c f)")
        acc_sq_f = acc_sq[:].rearrange("p c f -> p (c f)")
        n_fl = 8 * 136
        cols = [(c, min(128, n_fl - 128 * c)) for c in range((n_fl + 127) // 128)]
        assert len(cols) <= 9
        for col, mc in cols:
            nc.tensor.matmul(ps_t[0:mc, col:col + 1],
                             acc_lo_f[:, 128 * col:128 * col + mc], ones[:])
            nc.tensor.matmul(ps_t[0:mc, 9 + col:10 + col],
                             acc_sq_f[:, 128 * col:128 * col + mc], ones[:])
        res = sb1.tile([128, 18], f32)
        nc.vector.tensor_copy(res[:], ps_t[:])
        nc.sync.dma_start(OUT, res[:])

    nc.compile()
    return nc


def _unpack_stats(stats_list):
    """stats[p, col] (col<9: lo, col>=9: sq) holds flat index 128*col + p of
    the [8, 136] (A:72 | B:64) accumulator. Returns summed s1A, s2A, s1B, s2B."""
    n_fl = 8 * 136
    lo_fl = np.zeros(n_fl, np.float64)
    sq_fl = np.zeros(n_fl, np.float64)
    for st in stats_list:
        st = np.asarray(st, np.float64)
        for col in range((n_fl + 127) // 128):
            mc = min(128, n_fl - 128 * col)
            lo_fl[128 * col:128 * col + mc] += st[0:mc, col]
            sq_fl[128 * col:128 * col + mc] += st[0:mc, 9 + col]
    lo2 = lo_fl.reshape(8, 136)
    sq2 = sq_fl.reshape(8, 136)
    return (lo2[:, 0:72].sum(0), sq2[:, 0:72].sum(0),
            lo2[:, 72:136].sum(0), sq2[:, 72:136].sum(0))


def _build_main(rows):
    """NEFF 2: the full forward pass given pre-folded weights."""
    import concourse.mybir as mybir
    import concourse.tile as tile
    from concourse import bacc
    from concourse.masks import make_identity

    f32 = mybir.dt.float32
    f32r = mybir.dt.float32r
    bf16 = mybir.dt.bfloat16
    Ln = mybir.ActivationFunctionType.Ln
    Relu = mybir.ActivationFunctionType.Relu
    Tanh = mybir.ActivationFunctionType.Tanh
    n_grp = rows // 1024
    n_it = rows // 512
    n_b4 = rows // 2048   # 4-iteration blocks

    nc = bacc.Bacc("TRN2", target_bir_lowering=False, debug=False,
                   num_devices=N_CORES)
    X = nc.dram_tensor("x", [rows, 192], f32, kind="ExternalInput").ap()
    Z = nc.dram_tensor("z", [512, rows], bf16, kind="ExternalInput").ap()
    WZ = nc.dram_tensor("wz", [4, 128, 96], bf16, kind="ExternalInput").ap()
    WX = nc.dram_tensor("wx", [4, NX, 96], f32r, kind="ExternalInput").ap()
    WH = nc.dram_tensor("wh", [4, 96, 64], f32r, kind="ExternalInput").ap()
    B2 = nc.dram_tensor("b2", [64, 1], f32, kind="ExternalInput").ap()
    Y = nc.dram_tensor("y", [rows, 64], f32, kind="ExternalOutput").ap()

    with tile.TileContext(nc) as tc, ExitStack() as ctx:
        cst = ctx.enter_context(tc.tile_pool(name="cst", bufs=1))
        ident = cst.tile([128, 128], f32)
        make_identity(nc, ident[:])
        wz_sb = cst.tile([128, 4, 96], bf16)
        nc.sync.dma_start(wz_sb[:], WZ.rearrange("g k m -> k g m"))
        wx_sb = cst.tile([NX, 4, 96], f32r)
        nc.sync.dma_start(wx_sb[:], WX.rearrange("g k m -> k g m"))
        wh_sb = cst.tile([96, 4, 64], f32r)
        nc.sync.dma_start(wh_sb[:], WH.rearrange("g k m -> k g m"))
        b2_sb = cst.tile([64, 1], f32)
        nc.sync.dma_start(b2_sb[:], B2)
        xT = cst.tile([NX, n_it, 512], f32r)  # resident log1p(x)^T (+ones row)

        # ---- Phase A: build xT ----
        with tc.tile_pool(name="pha", bufs=3) as pha, \
             tc.tile_pool(name="psA", bufs=2, space="PSUM") as psA:
            Xv = X.rearrange("(g c p) f -> g p c f", p=128, c=8)
            for g in range(n_grp):
                tA = pha.tile([128, 8, 72], f32, tag="tA")
                nc.sync.dma_start(tA[:], Xv[g, :, :, 0:72])
                tB = pha.tile([128, 8, 64], f32, tag="tB")
                nc.sync.dma_start(tB[:], Xv[g, :, :, 128:192])
                lo = pha.tile([128, 8, NX], f32, tag="lo")
                nc.scalar.activation(lo[:, :, 0:1], tA[:, :, 0:1], Ln, bias=1.0)
                nc.scalar.activation(lo[:, :, 1:9], tA[:, :, 64:72], Ln, bias=1.0)
                nc.scalar.activation(lo[:, :, 9:73], tB[:], Ln, bias=1.0)
                nc.vector.memset(lo[:, :, 73:74], 1.0)
                for half in range(2):
                    pt = psA.tile([NX, 4, 128], f32, tag="pt")
                    for cc in range(4):
                        c = half * 4 + cc
                        nc.tensor.transpose(pt[:, cc, :], lo[:, c, :], ident[:])
                    nc.vector.tensor_copy(
                        xT[:, 2 * g + half, :],
                        pt[:].rearrange("p c f -> p (c f)"))

        # ---- Phase B: main loop ----
        with tc.tile_pool(name="zsp", bufs=2) as zsp, \
             tc.tile_pool(name="hsp", bufs=6) as hsp, \
             tc.tile_pool(name="ysp", bufs=3) as ysp, \
             tc.tile_pool(name="ystgp", bufs=2) as ystgp, \
             tc.tile_pool(name="psH", bufs=3, space="PSUM") as psH, \
             tc.tile_pool(name="psY", bufs=2, space="PSUM") as psY:
            Yv = Y.rearrange("(i c r) (a s) -> i a r c s", c=64, r=32, a=2,
                             s=32)
            for b4 in range(n_b4):
                zs4 = []
                for g in range(4):
                    zt = zsp.tile([128, 2048], bf16, tag=f"zs{g}")
                    eng = nc.sync if g % 2 == 0 else nc.scalar
                    eng.dma_start(
                        zt[:],
                        Z[128 * g:128 * (g + 1), b4 * 2048:(b4 + 1) * 2048])
                    zs4.append(zt)
                ystg = ystgp.tile([64, 4, 512], f32, tag="ystg")
                for i4 in range(4):
                    it = 4 * b4 + i4
                    hs_tiles = []
                    for g in range(4):
                        ph = psH.tile([96, 512], f32, tag="ph")
                        nc.tensor.matmul(ph[:], wz_sb[:, g, :],
                                         zs4[g][:, 512 * i4:512 * (i4 + 1)],
                                         start=True, stop=False)
                        nc.tensor.matmul(ph[:], wx_sb[:, g, :], xT[:, it, :],
                                         start=False, stop=True)
                        ht = hsp.tile([96, 512], f32r, tag="hs")
                        if g < 2:
                            nc.scalar.activation(ht[:], ph[:], Relu)
                        else:
                            nc.vector.tensor_scalar_max(ht[:], ph[:], 0.0)
                        hs_tiles.append(ht)
                    py = psY.tile([64, 512], f32, tag="py")
                    for g in range(4):
                        nc.tensor.matmul(py[:], wh_sb[:, g, :],
                                         hs_tiles[g][:],
                                         start=(g == 0), stop=(g == 3))
                    ysb = ysp.tile([64, 512], f32, tag="ysb")
                    nc.scalar.activation(ysb[:], py[:], Tanh, bias=b2_sb[:])
                    y12 = ysp.tile([64, 512], f32, tag="y12")
                    nc.vector.tensor_scalar_mul(y12[:], ysb[:], 12.0)
                    nc.vector.transpose(ystg[:, i4, :], y12[:])
                for a in range(2):
                    nc.sync.dma_start(
                        Yv[b4, a],
                        ystg[32 * a:32 * a + 32, :, :].rearrange(
                            "r c (q s) -> r (c q) s", s=32))

    nc.compile()
    return nc


def _get_modules(rows=SHARD):
    key = ("mods", rows)
    if key not in _cache:
        _cache[key] = (_build_stats(rows), _build_main(rows))
    return _cache[key]


def _fold_weights(stats_list, W1, b1, W2, b2, rows_total):
    """Combine per-core stats, compute mu/sd, fold standardization into W1/b1,
    and build the device weight layouts."""
    import ml_dtypes

    s1A, s2A, s1B, s2B = _unpack_stats(stats_list)
    n = float(rows_total)
    mu_root, mu_par, mu_own = s1A[0] / n, s1A[64:72] / n, s1B / n
    var = lambda s1, s2: (s2 - s1 * s1 / n) / (n - 1.0)
    sd_root = np.sqrt(var(s1A[0], s2A[0]))
    sd_par = np.sqrt(var(s1A[64:72], s2A[64:72]))
    sd_own = np.sqrt(var(s1B, s2B))

    W1 = np.asarray(W1, np.float64)
    b1 = np.asarray(b1, np.float64)
    W2 = np.asarray(W2, np.float64)
    b2 = np.asarray(b2, np.float64)
    Wz = W1[:, :, 0:8]
    Wown, Wpar, Wroot = W1[:, :, 8], W1[:, :, 9], W1[:, :, 10]
    par_idx = np.arange(NN) // 8
    Wown_f = Wown / sd_own[:, None]
    Wpar_f = Wpar / sd_par[par_idx][:, None]
    Wroot_f = Wroot / sd_root
    b1_f = (b1 - Wown * (mu_own / sd_own)[:, None]
            - Wpar * (mu_par / sd_par)[par_idx][:, None]
            - Wroot * (mu_root / sd_root))

    WZh = np.zeros((4, 128, 96), np.float32)
    WXh = np.zeros((4, NX, 96), np.float32)
    WHh = np.zeros((4, 96, 64), np.float32)
    for g in range(4):
        for nl in range(16):
            n_g = 16 * g + nl
            WZh[g, 8 * nl:8 * nl + 8, 6 * nl:6 * nl + 6] = Wz[n_g].T
            WXh[g, 0, 6 * nl:6 * nl + 6] = Wroot_f[n_g]
            WXh[g, 1 + n_g // 8, 6 * nl:6 * nl + 6] = Wpar_f[n_g]
            WXh[g, 9 + n_g, 6 * nl:6 * nl + 6] = Wown_f[n_g]
            WXh[g, 73, 6 * nl:6 * nl + 6] = b1_f[n_g]
            WHh[g, 6 * nl:6 * nl + 6, n_g] = 0.1 * W2[n_g, 0, :]
    B2h = (0.1 * b2).astype(np.float32).reshape(64, 1)
    return WZh.astype(ml_dtypes.bfloat16), WXh, WHh, B2h


def _prep_z(Z, shard):
    """Per-core shard of Z, cast to bf16 and transposed to [512, shard]."""
    import ml_dtypes
    n_cores = Z.shape[0] // shard
    outs = [np.empty((512, shard), ml_dtypes.bfloat16) for _ in range(n_cores)]
    def prep(si):
        s, i = divmod(si, 4)
        blk = shard // 4
        outs[s][:, i * blk:(i + 1) * blk] = \
            Z[s * shard + i * blk:s * shard + (i + 1) * blk].T
    with ThreadPoolExecutor(16) as ex:
        list(ex.map(prep, range(n_cores * 4)))
    return outs


def kernel(**inputs):
    from concourse.bass_utils import run_bass_kernel_spmd

    X = np.ascontiguousarray(
        np.asarray(inputs["X_1tol"], np.float32).reshape(-1, 192))
    rows_total = X.shape[0]
    shard = rows_total // N_CORES
    Zt = _prep_z(np.asarray(inputs["Z_l_next"], np.float32), shard)
    nc_stats, nc_main = _get_modules(shard)
    core_ids = list(range(N_CORES))

    in1 = [{"x": X[s * shard:(s + 1) * shard]} for s in range(N_CORES)]
    r1 = run_bass_kernel_spmd(nc_stats, in1, core_ids=core_ids)
    stats_list = [r1.results[s]["stats"] for s in range(N_CORES)]

    WZh, WXh, WHh, B2h = _fold_weights(
        stats_list, inputs["W1"], inputs["b1"], inputs["W2"], inputs["b2"],
        rows_total)

    in2 = [{"x": X[s * shard:(s + 1) * shard],
            "z": Zt[s],
            "wz": WZh, "wx": WXh, "wh": WHh, "b2": B2h}
           for s in range(N_CORES)]
    r2 = run_bass_kernel_spmd(nc_main, in2, core_ids=core_ids)
    out = np.concatenate([r2.results[s]["y"] for s in range(N_CORES)], axis=0)
    return out.astype(np.float32)


# revision 21
# speedup vs baseline: 1.6620x; 1.0358x over previous
"""Trainium2 Bass kernel for nn_BranchMarkovLayer (gnn_message_passing).

Computation (per batch row b, node n of 64):
    data[b,n,:] = [ Zc[b,n,0:8], std(log1p(own[b,n])), std(log1p(par[b,n//8])),
                    std(log1p(root[b])) ]                       (11 features)
    h = relu(W1[n] @ data + b1[n]);  y = W2[n] @ h + b2[n]      (11 -> 6 -> 1)
    out = -12 + 24*sigmoid(0.2*y) = 12*tanh(0.1*(W2' h + b2'))  (W2' = 0.1*W2)

Standardization (mean/std over the FULL batch, ddof=1) is folded into W1/b1 on
the host, given per-column sums/sumsq computed on-device by a small stats NEFF.

Sharding: pure data-parallel over the batch axis across 8 NeuronCores.

Main NEFF per core (shard = 16384 rows):
  Phase A: load X cols [0:72) and [128:192), log1p on ACT, PE-transpose the
           74-feature blocks (root, par x8, own x64, ones) into a resident
           xT [74, 16384] float32r SBUF tensor.
  Phase B: per 512-row tile: Z (pre-cast to bf16 and pre-transposed per shard
           on the host) is loaded as [128f, 2048b] tiles with plain large-burst
           DMAs (one per 16-node group per 4 iterations). Block-diagonal bf16
           matmuls for
           the layer-1 z-part accumulate with a float32r matmul for the x-part
           (+bias via the ones row) in PSUM [96, 512]; relu (split ACT/DVE)
           writes float32r; layer-2 float32r matmuls accumulate into y psum
           [64, 512]; tanh(+b2 bias) on ACT; x12 and a 32x32 block-transpose
           on DVE into a [64, 2048] staging tile; strided DMA store (128B
           bursts) to the natural [B, 64] layout every 4 iterations.
"""

import numpy as np
from concurrent.futures import ThreadPoolExecutor
from contextlib import ExitStack

N_CORES = 8
B_FULL = 131072
SHARD = B_FULL // N_CORES  # 16384
NN = 64      # nodes
NX = 74      # xT rows: root(1) + par(8) + own(64) + ones(1)

# A-chunks of the flattened [8, 72] free block, then B-chunks of [8, 64]
CHUNKS_A = [(0, 0, 128), (1, 128, 128), (2, 256, 128), (3, 384, 128),
            (4, 512, 64)]
CHUNKS_B = [(5, 0, 128), (6, 128, 128), (7, 256, 128), (8, 384, 128)]

_cache = {}


def _build_stats(rows):
    """NEFF 1: per-column sums and sum-of-squares of log1p over X columns
    [0:72) ("A": root@0, par@64..71) and [128:192) ("B": own).

    Per 1024-row group: log1p + square on ACT, accumulate into SBUF via DVE
    adds.  At the end, per-column sums via fp32 matmuls against a ones vector
    (contraction over the 128 batch partitions)."""
    import concourse.mybir as mybir
    import concourse.tile as tile
    from concourse import bacc

    f32 = mybir.dt.float32
    Ln = mybir.ActivationFunctionType.Ln
    Sq = mybir.ActivationFunctionType.Square
    n_grp = rows // 1024

    nc = bacc.Bacc("TRN2", target_bir_lowering=False, debug=False,
                   num_devices=N_CORES)
    X = nc.dram_tensor("x", [rows, 192], f32, kind="ExternalInput").ap()
    OUT = nc.dram_tensor("stats", [128, 18], f32, kind="ExternalOutput").ap()

    with tile.TileContext(nc) as tc, ExitStack() as ctx:
        sb = ctx.enter_context(tc.tile_pool(name="sb", bufs=3))
        sb1 = ctx.enter_context(tc.tile_pool(name="sb1", bufs=1))
        ps = ctx.enter_context(tc.tile_pool(name="ps", bufs=1, space="PSUM"))

        ones = sb1.tile([128, 1], f32)
        nc.vector.memset(ones[:], 1.0)
        acc_lo = sb1.tile([128, 8, 136], f32)   # [..., 0:72) = A, [72:136) = B
        nc.vector.memset(acc_lo[:], 0.0)
        acc_sq = sb1.tile([128, 8, 136], f32)
        nc.vector.memset(acc_sq[:], 0.0)

        Xv = X.rearrange("(g c p) f -> g p c f", p=128, c=8)
        for g in range(n_grp):
            tA = sb.tile([128, 8, 72], f32, tag="tA")
            nc.sync.dma_start(tA[:], Xv[g, :, :, 0:72])
            tB = sb.tile([128, 8, 64], f32, tag="tB")
            nc.sync.dma_start(tB[:], Xv[g, :, :, 128:192])
            lo = sb.tile([128, 8, 136], f32, tag="lo")
            nc.scalar.activation(lo[:, :, 0:72], tA[:], Ln, bias=1.0)
            nc.scalar.activation(lo[:, :, 72:136], tB[:], Ln, bias=1.0)
            sq = sb.tile([128, 8, 136], f32, tag="sq")
            nc.scalar.activation(sq[:], lo[:], Sq)
            nc.vector.tensor_add(acc_lo[:], acc_lo[:], lo[:])
            nc.vector.tensor_add(acc_sq[:], acc_sq[:], sq[:])

        # Final per-column sums: contract the 128 batch partitions on PE.
        # Flat layout: [8, 136] -> A cols at c*136+f (f<72), B at c*136+72+f.
        ps_t = ps.tile([128, 18], f32)
        acc_lo_f = acc_lo[:].rearrange("p c f -> p (c f)")
        acc_sq_f = acc_sq[:].rearrange("p c f -> p (c f)")
        n_fl = 8 * 136
        cols = [(c, min(128, n_fl - 128 * c)) for c in range((n_fl + 127) // 128)]
        assert len(cols) <= 9
        for col, mc in cols:
            nc.tensor.matmul(ps_t[0:mc, col:col + 1],
                             acc_lo_f[:, 128 * col:128 * col + mc], ones[:])
            nc.tensor.matmul(ps_t[0:mc, 9 + col:10 + col],
                             acc_sq_f[:, 128 * col:128 * col + mc], ones[:])
        res = sb1.tile([128, 18], f32)
        nc.vector.tensor_copy(res[:], ps_t[:])
        nc.sync.dma_start(OUT, res[:])

    nc.compile()
    return nc


def _unpack_stats(stats_list):
    """stats[p, col] (col<9: lo, col>=9: sq) holds flat index 128*col + p of
    the [8, 136] (A:72 | B:64) accumulator. Returns summed s1A, s2A, s1B, s2B."""
    n_fl = 8 * 136
    lo_fl = np.zeros(n_fl, np.float64)
    sq_fl = np.zeros(n_fl, np.float64)
    for st in stats_list:
        st = np.asarray(st, np.float64)
        for col in range((n_fl + 127) // 128):
            mc = min(128, n_fl - 128 * col)
            lo_fl[128 * col:128 * col + mc] += st[0:mc, col]
            sq_fl[128 * col:128 * col + mc] += st[0:mc, 9 + col]
    lo2 = lo_fl.reshape(8, 136)
    sq2 = sq_fl.reshape(8, 136)
    return (lo2[:, 0:72].sum(0), sq2[:, 0:72].sum(0),
            lo2[:, 72:136].sum(0), sq2[:, 72:136].sum(0))


def _build_main(rows):
    """NEFF 2: the full forward pass given pre-folded weights."""
    import concourse.mybir as mybir
    import concourse.tile as tile
    from concourse import bacc
    from concourse.masks import make_identity

    f32 = mybir.dt.float32
    f32r = mybir.dt.float32r
    bf16 = mybir.dt.bfloat16
    Ln = mybir.ActivationFunctionType.Ln
    Relu = mybir.ActivationFunctionType.Relu
    Tanh = mybir.ActivationFunctionType.Tanh
    n_grp = rows // 1024
    n_it = rows // 512
    n_b4 = rows // 2048   # 4-iteration blocks

    nc = bacc.Bacc("TRN2", target_bir_lowering=False, debug=False,
                   num_devices=N_CORES)
    X = nc.dram_tensor("x", [rows, 192], f32, kind="ExternalInput").ap()
    Z = nc.dram_tensor("z", [512, rows], bf16, kind="ExternalInput").ap()
    WZ = nc.dram_tensor("wz", [4, 128, 96], bf16, kind="ExternalInput").ap()
    WX = nc.dram_tensor("wx", [4, NX, 96], f32r, kind="ExternalInput").ap()
    WH = nc.dram_tensor("wh", [4, 96, 64], f32r, kind="ExternalInput").ap()
    B2 = nc.dram_tensor("b2", [64, 1], f32, kind="ExternalInput").ap()
    Y = nc.dram_tensor("y", [rows, 64], f32, kind="ExternalOutput").ap()

    with tile.TileContext(nc) as tc, ExitStack() as ctx:
        cst = ctx.enter_context(tc.tile_pool(name="cst", bufs=1))
        ident = cst.tile([128, 128], f32)
        make_identity(nc, ident[:])
        wz_sb = cst.tile([128, 4, 96], bf16)
        nc.sync.dma_start(wz_sb[:], WZ.rearrange("g k m -> k g m"))
        wx_sb = cst.tile([NX, 4, 96], f32r)
        nc.sync.dma_start(wx_sb[:], WX.rearrange("g k m -> k g m"))
        wh_sb = cst.tile([96, 4, 64], f32r)
        nc.sync.dma_start(wh_sb[:], WH.rearrange("g k m -> k g m"))
        b2_sb = cst.tile([64, 1], f32)
        nc.sync.dma_start(b2_sb[:], B2)
        xT = cst.tile([NX, n_it, 512], f32r)  # resident log1p(x)^T (+ones row)

        # ---- Phase A: build xT ----
        with tc.tile_pool(name="pha", bufs=3) as pha, \
             tc.tile_pool(name="psA", bufs=2, space="PSUM") as psA:
            Xv = X.rearrange("(g c p) f -> g p c f", p=128, c=8)
            for g in range(n_grp):
                tA = pha.tile([128, 8, 72], f32, tag="tA")
                nc.sync.dma_start(tA[:], Xv[g, :, :, 0:72])
                tB = pha.tile([128, 8, 64], f32, tag="tB")
                nc.sync.dma_start(tB[:], Xv[g, :, :, 128:192])
                lo = pha.tile([128, 8, NX], f32, tag="lo")
                nc.scalar.activation(lo[:, :, 0:1], tA[:, :, 0:1], Ln, bias=1.0)
                nc.scalar.activation(lo[:, :, 1:9], tA[:, :, 64:72], Ln, bias=1.0)
                nc.scalar.activation(lo[:, :, 9:73], tB[:], Ln, bias=1.0)
                nc.vector.memset(lo[:, :, 73:74], 1.0)
                for half in range(2):
                    pt = psA.tile([NX, 4, 128], f32, tag="pt")
                    for cc in range(4):
                        c = half * 4 + cc
                        nc.tensor.transpose(pt[:, cc, :], lo[:, c, :], ident[:])
                    nc.vector.tensor_copy(
                        xT[:, 2 * g + half, :],
                        pt[:].rearrange("p c f -> p (c f)"))

        # ---- Phase B: main loop ----
        with tc.tile_pool(name="zsp", bufs=2) as zsp, \
             tc.tile_pool(name="hsp", bufs=6) as hsp, \
             tc.tile_pool(name="ysp", bufs=3) as ysp, \
             tc.tile_pool(name="ystgp", bufs=2) as ystgp, \
             tc.tile_pool(name="psH", bufs=3, space="PSUM") as psH, \
             tc.tile_pool(name="psY", bufs=2, space="PSUM") as psY:
            Yv = Y.rearrange("(i c r) (a s) -> i a r c s", c=64, r=32, a=2,
                             s=32)
            for b4 in range(n_b4):
                zs4 = []
                for g in range(4):
                    zt = zsp.tile([128, 2048], bf16, tag=f"zs{g}")
                    eng = nc.sync if g % 2 == 0 else nc.scalar
                    eng.dma_start(
                        zt[:],
                        Z[128 * g:128 * (g + 1), b4 * 2048:(b4 + 1) * 2048])
                    zs4.append(zt)
                ystg = ystgp.tile([64, 4, 512], f32, tag="ystg")
                for i4 in range(4):
                    it = 4 * b4 + i4
                    hs_tiles = []
                    for g in range(4):
                        ph = psH.tile([96, 512], f32, tag="ph")
                        nc.tensor.matmul(ph[:], wz_sb[:, g, :],
                                         zs4[g][:, 512 * i4:512 * (i4 + 1)],
                                         start=True, stop=False)
                        nc.tensor.matmul(ph[:], wx_sb[:, g, :], xT[:, it, :],
                                         start=False, stop=True)
                        ht = hsp.tile([96, 512], f32r, tag="hs")
                        if g < 2:
                            nc.scalar.activation(ht[:], ph[:], Relu)
                        else:
                            nc.vector.tensor_scalar_max(ht[:], ph[:], 0.0)
                        hs_tiles.append(ht)
                    py = psY.tile([64, 512], f32, tag="py")
                    for g in range(4):
                        nc.tensor.matmul(py[:], wh_sb[:, g, :],
                                         hs_tiles[g][:],
                                         start=(g == 0), stop=(g == 3))
                    ysb = ysp.tile([64, 512], f32, tag="ysb")
                    nc.scalar.activation(ysb[:], py[:], Tanh, bias=b2_sb[:])
                    y12 = ysp.tile([64, 512], f32, tag="y12")
                    nc.vector.tensor_scalar_mul(y12[:], ysb[:], 12.0)
                    nc.vector.transpose(ystg[:, i4, :], y12[:])
                for a in range(2):
                    nc.sync.dma_start(
                        Yv[b4, a],
                        ystg[32 * a:32 * a + 32, :, :].rearrange(
                            "r c (q s) -> r (c q) s", s=32))

    nc.compile()
    return nc


def _get_modules(rows=SHARD):
    key = ("mods", rows)
    if key not in _cache:
        _cache[key] = (_build_stats(rows), _build_main(rows))
    return _cache[key]


def _fold_weights(stats_list, W1, b1, W2, b2, rows_total):
    """Combine per-core stats, compute mu/sd, fold standardization into W1/b1,
    and build the device weight layouts."""
    import ml_dtypes

    s1A, s2A, s1B, s2B = _unpack_stats(stats_list)
    n = float(rows_total)
    mu_root, mu_par, mu_own = s1A[0] / n, s1A[64:72] / n, s1B / n
    var = lambda s1, s2: (s2 - s1 * s1 / n) / (n - 1.0)
    sd_root = np.sqrt(var(s1A[0], s2A[0]))
    sd_par = np.sqrt(var(s1A[64:72], s2A[64:72]))
    sd_own = np.sqrt(var(s1B, s2B))

    W1 = np.asarray(W1, np.float64)
    b1 = np.asarray(b1, np.float64)
    W2 = np.asarray(W2, np.float64)
    b2 = np.asarray(b2, np.float64)
    Wz = W1[:, :, 0:8]
    Wown, Wpar, Wroot = W1[:, :, 8], W1[:, :, 9], W1[:, :, 10]
    par_idx = np.arange(NN) // 8
    Wown_f = Wown / sd_own[:, None]
    Wpar_f = Wpar / sd_par[par_idx][:, None]
    Wroot_f = Wroot / sd_root
    b1_f = (b1 - Wown * (mu_own / sd_own)[:, None]
            - Wpar * (mu_par / sd_par)[par_idx][:, None]
            - Wroot * (mu_root / sd_root))

    WZh = np.zeros((4, 128, 96), np.float32)
    WXh = np.zeros((4, NX, 96), np.float32)
    WHh = np.zeros((4, 96, 64), np.float32)
    for g in range(4):
        for nl in range(16):
            n_g = 16 * g + nl
            WZh[g, 8 * nl:8 * nl + 8, 6 * nl:6 * nl + 6] = Wz[n_g].T
            WXh[g, 0, 6 * nl:6 * nl + 6] = Wroot_f[n_g]
            WXh[g, 1 + n_g // 8, 6 * nl:6 * nl + 6] = Wpar_f[n_g]
            WXh[g, 9 + n_g, 6 * nl:6 * nl + 6] = Wown_f[n_g]
            WXh[g, 73, 6 * nl:6 * nl + 6] = b1_f[n_g]
            WHh[g, 6 * nl:6 * nl + 6, n_g] = 0.1 * W2[n_g, 0, :]
    B2h = (0.1 * b2).astype(np.float32).reshape(64, 1)
    return WZh.astype(ml_dtypes.bfloat16), WXh, WHh, B2h


def _prep_z(Z, shard):
    """Per-core shard of Z, cast to bf16 and transposed to [512, shard]."""
    import ml_dtypes
    n_cores = Z.shape[0] // shard
    outs = [np.empty((512, shard), ml_dtypes.bfloat16) for _ in range(n_cores)]
    def prep(si):
        s, i = divmod(si, 4)
        blk = shard // 4
        outs[s][:, i * blk:(i + 1) * blk] = \
            Z[s * shard + i * blk:s * shard + (i + 1) * blk].T
    with ThreadPoolExecutor(16) as ex:
        list(ex.map(prep, range(n_cores * 4)))
    return outs


def kernel(**inputs):
    from concourse.bass_utils import run_bass_kernel_spmd

    X = np.ascontiguousarray(
        np.asarray(inputs["X_1tol"], np.float32).reshape(-1, 192))
    rows_total = X.shape[0]
    shard = rows_total // N_CORES
    Zt = _prep_z(np.asarray(inputs["Z_l_next"], np.float32), shard)
    nc_stats, nc_main = _get_modules(shard)
    core_ids = list(range(N_CORES))

    in1 = [{"x": X[s * shard:(s + 1) * shard]} for s in range(N_CORES)]
    r1 = run_bass_kernel_spmd(nc_stats, in1, core_ids=core_ids)
    stats_list = [r1.results[s]["stats"] for s in range(N_CORES)]

    WZh, WXh, WHh, B2h = _fold_weights(
        stats_list, inputs["W1"], inputs["b1"], inputs["W2"], inputs["b2"],
        rows_total)

    in2 = [{"x": X[s * shard:(s + 1) * shard],
            "z": Zt[s],
            "wz": WZh, "wx": WXh, "wh": WHh, "b2": B2h}
           for s in range(N_CORES)]
    r2 = run_bass_kernel_spmd(nc_main, in2, core_ids=core_ids)
    out = np.concatenate([r2.results[s]["y"] for s in range(N_CORES)], axis=0)
    return out.astype(np.float32)


# revision 23
# speedup vs baseline: 1.7280x; 1.0397x over previous
"""Trainium2 Bass kernel for nn_BranchMarkovLayer (gnn_message_passing).

Computation (per batch row b, node n of 64):
    data[b,n,:] = [ Zc[b,n,0:8], std(log1p(own[b,n])), std(log1p(par[b,n//8])),
                    std(log1p(root[b])) ]                       (11 features)
    h = relu(W1[n] @ data + b1[n]);  y = W2[n] @ h + b2[n]      (11 -> 6 -> 1)
    out = -12 + 24*sigmoid(0.2*y) = 12*tanh(0.1*(W2' h + b2'))  (W2' = 0.1*W2)

Standardization (mean/std over the FULL batch, ddof=1) is folded into W1/b1 on
the host, given per-column sums/sumsq computed on-device by a small stats NEFF.

Sharding: pure data-parallel over the batch axis across 8 NeuronCores.

Main NEFF per core (shard = 16384 rows):
  Phase A: load X cols [0:72) and [128:192), log1p on ACT, PE-transpose the
           74-feature blocks (root, par x8, own x64, ones) into a resident
           xT [74, 16384] float32r SBUF tensor.
  Phase B: per 512-row tile: Z (pre-cast to bf16 and pre-transposed per shard
           on the host) is loaded as [128f, 2048b] tiles with plain large-burst
           DMAs (one per 16-node group per 4 iterations). Block-diagonal bf16
           matmuls for
           the layer-1 z-part accumulate with a float32r matmul for the x-part
           (+bias via the ones row) in PSUM [96, 512]; relu (split ACT/DVE)
           writes float32r; layer-2 float32r matmuls accumulate into y psum
           [64, 512]; tanh(+b2 bias) on ACT; x12 and a 32x32 block-transpose
           on DVE into a [64, 2048] staging tile; strided DMA store (128B
           bursts) to the natural [B, 64] layout every 4 iterations.
"""

import numpy as np
from concurrent.futures import ThreadPoolExecutor
from contextlib import ExitStack

N_CORES = 8
B_FULL = 131072
SHARD = B_FULL // N_CORES  # 16384
NN = 64      # nodes
NX = 74      # xT rows: root(1) + par(8) + own(64) + ones(1)

# A-chunks of the flattened [8, 72] free block, then B-chunks of [8, 64]
CHUNKS_A = [(0, 0, 128), (1, 128, 128), (2, 256, 128), (3, 384, 128),
            (4, 512, 64)]
CHUNKS_B = [(5, 0, 128), (6, 128, 128), (7, 256, 128), (8, 384, 128)]

_cache = {}


def _build_stats(rows):
    """NEFF 1: per-column sums and sum-of-squares of log1p over X columns
    [0:72) ("A": root@0, par@64..71) and [128:192) ("B": own).

    Per 1024-row group: log1p + square on ACT, accumulate into SBUF via DVE
    adds.  At the end, per-column sums via fp32 matmuls against a ones vector
    (contraction over the 128 batch partitions)."""
    import concourse.mybir as mybir
    import concourse.tile as tile
    from concourse import bacc

    f32 = mybir.dt.float32
    Ln = mybir.ActivationFunctionType.Ln
    Sq = mybir.ActivationFunctionType.Square
    n_grp = rows // 1024

    nc = bacc.Bacc("TRN2", target_bir_lowering=False, debug=False,
                   num_devices=N_CORES)
    X = nc.dram_tensor("x", [rows, 192], f32, kind="ExternalInput").ap()
    OUT = nc.dram_tensor("stats", [128, 18], f32, kind="ExternalOutput").ap()

    with tile.TileContext(nc) as tc, ExitStack() as ctx:
        sb = ctx.enter_context(tc.tile_pool(name="sb", bufs=3))
        sb1 = ctx.enter_context(tc.tile_pool(name="sb1", bufs=1))
        ps = ctx.enter_context(tc.tile_pool(name="ps", bufs=1, space="PSUM"))

        ones = sb1.tile([128, 1], f32)
        nc.vector.memset(ones[:], 1.0)
        acc_lo = sb1.tile([128, 8, 136], f32)   # [..., 0:72) = A, [72:136) = B
        nc.vector.memset(acc_lo[:], 0.0)
        acc_sq = sb1.tile([128, 8, 136], f32)
        nc.vector.memset(acc_sq[:], 0.0)

        Xv = X.rearrange("(g c p) f -> g p c f", p=128, c=8)
        for g in range(n_grp):
            tA = sb.tile([128, 8, 72], f32, tag="tA")
            nc.sync.dma_start(tA[:], Xv[g, :, :, 0:72])
            tB = sb.tile([128, 8, 64], f32, tag="tB")
            nc.sync.dma_start(tB[:], Xv[g, :, :, 128:192])
            lo = sb.tile([128, 8, 136], f32, tag="lo")
            nc.scalar.activation(lo[:, :, 0:72], tA[:], Ln, bias=1.0)
            nc.scalar.activation(lo[:, :, 72:136], tB[:], Ln, bias=1.0)
            sq = sb.tile([128, 8, 136], f32, tag="sq")
            nc.scalar.activation(sq[:], lo[:], Sq)
            nc.vector.tensor_add(acc_lo[:], acc_lo[:], lo[:])
            nc.vector.tensor_add(acc_sq[:], acc_sq[:], sq[:])

        # Final per-column sums: contract the 128 batch partitions on PE.
        # Flat layout: [8, 136] -> A cols at c*136+f (f<72), B at c*136+72+f.
        ps_t = ps.tile([128, 18], f32)
        acc_lo_f = acc_lo[:].rearrange("p c f -> p (c f)")
        acc_sq_f = acc_sq[:].rearrange("p c f -> p (c f)")
        n_fl = 8 * 136
        cols = [(c, min(128, n_fl - 128 * c)) for c in range((n_fl + 127) // 128)]
        assert len(cols) <= 9
        for col, mc in cols:
            nc.tensor.matmul(ps_t[0:mc, col:col + 1],
                             acc_lo_f[:, 128 * col:128 * col + mc], ones[:])
            nc.tensor.matmul(ps_t[0:mc, 9 + col:10 + col],
                             acc_sq_f[:, 128 * col:128 * col + mc], ones[:])
        res = sb1.tile([128, 18], f32)
        nc.vector.tensor_copy(res[:], ps_t[:])
        nc.sync.dma_start(OUT, res[:])

    nc.compile()
    return nc


def _unpack_stats(stats_list):
    """stats[p, col] (col<9: lo, col>=9: sq) holds flat index 128*col + p of
    the [8, 136] (A:72 | B:64) accumulator. Returns summed s1A, s2A, s1B, s2B."""
    n_fl = 8 * 136
    lo_fl = np.zeros(n_fl, np.float64)
    sq_fl = np.zeros(n_fl, np.float64)
    for st in stats_list:
        st = np.asarray(st, np.float64)
        for col in range((n_fl + 127) // 128):
            mc = min(128, n_fl - 128 * col)
            lo_fl[128 * col:128 * col + mc] += st[0:mc, col]
            sq_fl[128 * col:128 * col + mc] += st[0:mc, 9 + col]
    lo2 = lo_fl.reshape(8, 136)
    sq2 = sq_fl.reshape(8, 136)
    return (lo2[:, 0:72].sum(0), sq2[:, 0:72].sum(0),
            lo2[:, 72:136].sum(0), sq2[:, 72:136].sum(0))


def _build_main(rows):
    """NEFF 2: the full forward pass given pre-folded weights."""
    import concourse.mybir as mybir
    import concourse.tile as tile
    from concourse import bacc
    from concourse.masks import make_identity

    f32 = mybir.dt.float32
    f32r = mybir.dt.float32r
    bf16 = mybir.dt.bfloat16
    Ln = mybir.ActivationFunctionType.Ln
    Relu = mybir.ActivationFunctionType.Relu
    Tanh = mybir.ActivationFunctionType.Tanh
    n_grp = rows // 1024
    n_it = rows // 512
    n_b4 = rows // 2048   # 4-iteration blocks

    nc = bacc.Bacc("TRN2", target_bir_lowering=False, debug=False,
                   num_devices=N_CORES)
    X = nc.dram_tensor("x", [rows, 192], f32, kind="ExternalInput").ap()
    Z = nc.dram_tensor("z", [512, rows], bf16, kind="ExternalInput").ap()
    WZ = nc.dram_tensor("wz", [4, 128, 96], bf16, kind="ExternalInput").ap()
    WX = nc.dram_tensor("wx", [4, NX, 96], f32r, kind="ExternalInput").ap()
    WH = nc.dram_tensor("wh", [4, 96, 64], f32r, kind="ExternalInput").ap()
    B2 = nc.dram_tensor("b2", [64, 1], f32, kind="ExternalInput").ap()
    Y = nc.dram_tensor("y", [rows, 64], f32, kind="ExternalOutput").ap()

    with tile.TileContext(nc) as tc, ExitStack() as ctx:
        cst = ctx.enter_context(tc.tile_pool(name="cst", bufs=1))
        ident = cst.tile([128, 128], f32)
        make_identity(nc, ident[:])
        wz_sb = cst.tile([128, 4, 96], bf16)
        nc.sync.dma_start(wz_sb[:], WZ.rearrange("g k m -> k g m"))
        wx_sb = cst.tile([NX, 4, 96], f32r)
        nc.sync.dma_start(wx_sb[:], WX.rearrange("g k m -> k g m"))
        wh_sb = cst.tile([96, 4, 64], f32r)
        nc.sync.dma_start(wh_sb[:], WH.rearrange("g k m -> k g m"))
        b2_sb = cst.tile([64, 1], f32)
        nc.sync.dma_start(b2_sb[:], B2)
        xT = cst.tile([NX, n_it, 512], f32r)  # resident log1p(x)^T (+ones row)

        # ---- Phase A: build xT ----
        with tc.tile_pool(name="pha", bufs=3) as pha, \
             tc.tile_pool(name="psA", bufs=2, space="PSUM") as psA:
            Xv = X.rearrange("(g c p) f -> g p c f", p=128, c=8)
            for g in range(n_grp):
                tA = pha.tile([128, 8, 192], f32, tag="tA")
                eng = nc.sync if g % 2 == 0 else nc.scalar
                eng.dma_start(tA[:], Xv[g])
                lo = pha.tile([128, 8, NX], f32, tag="lo")
                nc.scalar.activation(lo[:, :, 0:1], tA[:, :, 0:1], Ln, bias=1.0)
                nc.scalar.activation(lo[:, :, 1:9], tA[:, :, 64:72], Ln, bias=1.0)
                nc.scalar.activation(lo[:, :, 9:73], tA[:, :, 128:192], Ln,
                                     bias=1.0)
                nc.vector.memset(lo[:, :, 73:74], 1.0)
                for half in range(2):
                    pt = psA.tile([NX, 4, 128], f32, tag="pt")
                    for cc in range(4):
                        c = half * 4 + cc
                        nc.tensor.transpose(pt[:, cc, :], lo[:, c, :], ident[:])
                    nc.vector.tensor_copy(
                        xT[:, 2 * g + half, :],
                        pt[:].rearrange("p c f -> p (c f)"))

        # ---- Phase B: main loop ----
        with tc.tile_pool(name="zsp", bufs=2) as zsp, \
             tc.tile_pool(name="hsp", bufs=6) as hsp, \
             tc.tile_pool(name="ysp", bufs=3) as ysp, \
             tc.tile_pool(name="ystgp", bufs=2) as ystgp, \
             tc.tile_pool(name="psH", bufs=3, space="PSUM") as psH, \
             tc.tile_pool(name="psY", bufs=2, space="PSUM") as psY, \
             tc.tile_pool(name="psT", bufs=2, space="PSUM") as psT:
            Yv = Y.rearrange("(b ic p) n -> b p ic n", ic=16, p=128)
            for b4 in range(n_b4):
                zs4 = []
                for g in range(4):
                    zt = zsp.tile([128, 2048], bf16, tag=f"zs{g}")
                    eng = nc.sync if g % 2 == 0 else nc.scalar
                    eng.dma_start(
                        zt[:],
                        Z[128 * g:128 * (g + 1), b4 * 2048:(b4 + 1) * 2048])
                    zs4.append(zt)
                ystg = ystgp.tile([128, 4, 4, 64], f32, tag="ystg")
                for i4 in range(4):
                    it = 4 * b4 + i4
                    hs_tiles = []
                    for g in range(4):
                        ph = psH.tile([96, 512], f32, tag="ph")
                        nc.tensor.matmul(ph[:], wz_sb[:, g, :],
                                         zs4[g][:, 512 * i4:512 * (i4 + 1)],
                                         start=True, stop=False)
                        nc.tensor.matmul(ph[:], wx_sb[:, g, :], xT[:, it, :],
                                         start=False, stop=True)
                        ht = hsp.tile([96, 512], f32r, tag="hs")
                        if g < 2:
                            nc.scalar.activation(ht[:], ph[:], Relu)
                        else:
                            nc.vector.tensor_scalar_max(ht[:], ph[:], 0.0)
                        hs_tiles.append(ht)
                    py = psY.tile([64, 512], f32, tag="py")
                    for g in range(4):
                        nc.tensor.matmul(py[:], wh_sb[:, g, :],
                                         hs_tiles[g][:],
                                         start=(g == 0), stop=(g == 3))
                    ysb = ysp.tile([64, 512], f32, tag="ysb")
                    nc.scalar.activation(ysb[:], py[:], Tanh, bias=b2_sb[:])
                    pt = psT.tile([128, 4, 64], f32, tag="ptY")
                    for c in range(4):
                        nc.tensor.transpose(pt[:, c, :],
                                            ysb[:, 128 * c:128 * (c + 1)],
                                            ident[0:64, 0:64])
                    nc.vector.tensor_scalar_mul(ystg[:, i4, :, :], pt[:], 12.0)
                nc.scalar.dma_start(Yv[b4], ystg[:].rearrange(
                    "p i c n -> p (i c) n"))

    nc.compile()
    return nc


def _get_modules(rows=SHARD):
    key = ("mods", rows)
    if key not in _cache:
        _cache[key] = (_build_stats(rows), _build_main(rows))
    return _cache[key]


def _fold_weights(stats_list, W1, b1, W2, b2, rows_total):
    """Combine per-core stats, compute mu/sd, fold standardization into W1/b1,
    and build the device weight layouts."""
    import ml_dtypes

    s1A, s2A, s1B, s2B = _unpack_stats(stats_list)
    n = float(rows_total)
    mu_root, mu_par, mu_own = s1A[0] / n, s1A[64:72] / n, s1B / n
    var = lambda s1, s2: (s2 - s1 * s1 / n) / (n - 1.0)
    sd_root = np.sqrt(var(s1A[0], s2A[0]))
    sd_par = np.sqrt(var(s1A[64:72], s2A[64:72]))
    sd_own = np.sqrt(var(s1B, s2B))

    W1 = np.asarray(W1, np.float64)
    b1 = np.asarray(b1, np.float64)
    W2 = np.asarray(W2, np.float64)
    b2 = np.asarray(b2, np.float64)
    Wz = W1[:, :, 0:8]
    Wown, Wpar, Wroot = W1[:, :, 8], W1[:, :, 9], W1[:, :, 10]
    par_idx = np.arange(NN) // 8
    Wown_f = Wown / sd_own[:, None]
    Wpar_f = Wpar / sd_par[par_idx][:, None]
    Wroot_f = Wroot / sd_root
    b1_f = (b1 - Wown * (mu_own / sd_own)[:, None]
            - Wpar * (mu_par / sd_par)[par_idx][:, None]
            - Wroot * (mu_root / sd_root))

    WZh = np.zeros((4, 128, 96), np.float32)
    WXh = np.zeros((4, NX, 96), np.float32)
    WHh = np.zeros((4, 96, 64), np.float32)
    for g in range(4):
        for nl in range(16):
            n_g = 16 * g + nl
            WZh[g, 8 * nl:8 * nl + 8, 6 * nl:6 * nl + 6] = Wz[n_g].T
            WXh[g, 0, 6 * nl:6 * nl + 6] = Wroot_f[n_g]
            WXh[g, 1 + n_g // 8, 6 * nl:6 * nl + 6] = Wpar_f[n_g]
            WXh[g, 9 + n_g, 6 * nl:6 * nl + 6] = Wown_f[n_g]
            WXh[g, 73, 6 * nl:6 * nl + 6] = b1_f[n_g]
            WHh[g, 6 * nl:6 * nl + 6, n_g] = 0.1 * W2[n_g, 0, :]
    B2h = (0.1 * b2).astype(np.float32).reshape(64, 1)
    return WZh.astype(ml_dtypes.bfloat16), WXh, WHh, B2h


def _prep_z(Z, shard):
    """Per-core shard of Z, cast to bf16 and transposed to [512, shard]."""
    import ml_dtypes
    n_cores = Z.shape[0] // shard
    outs = [np.empty((512, shard), ml_dtypes.bfloat16) for _ in range(n_cores)]
    def prep(si):
        s, i = divmod(si, 4)
        blk = shard // 4
        outs[s][:, i * blk:(i + 1) * blk] = \
            Z[s * shard + i * blk:s * shard + (i + 1) * blk].T
    with ThreadPoolExecutor(16) as ex:
        list(ex.map(prep, range(n_cores * 4)))
    return outs


def kernel(**inputs):
    from concourse.bass_utils import run_bass_kernel_spmd

    X = np.ascontiguousarray(
        np.asarray(inputs["X_1tol"], np.float32).reshape(-1, 192))
    rows_total = X.shape[0]
    shard = rows_total // N_CORES
    Zt = _prep_z(np.asarray(inputs["Z_l_next"], np.float32), shard)
    nc_stats, nc_main = _get_modules(shard)
    core_ids = list(range(N_CORES))

    in1 = [{"x": X[s * shard:(s + 1) * shard]} for s in range(N_CORES)]
    r1 = run_bass_kernel_spmd(nc_stats, in1, core_ids=core_ids)
    stats_list = [r1.results[s]["stats"] for s in range(N_CORES)]

    WZh, WXh, WHh, B2h = _fold_weights(
        stats_list, inputs["W1"], inputs["b1"], inputs["W2"], inputs["b2"],
        rows_total)

    in2 = [{"x": X[s * shard:(s + 1) * shard],
            "z": Zt[s],
            "wz": WZh, "wx": WXh, "wh": WHh, "b2": B2h}
           for s in range(N_CORES)]
    r2 = run_bass_kernel_spmd(nc_main, in2, core_ids=core_ids)
    out = np.concatenate([r2.results[s]["y"] for s in range(N_CORES)], axis=0)
    return out.astype(np.float32)
